# revision 3
# baseline (speedup 1.0000x reference)
"""Trainium2 Bass kernel for the CementPINN MLP (dense_mlp, 8 cores).

Data-parallel: x [32768, 8] is sharded along batch into 8 shards of 4096
rows; MLP weights are replicated on every core.  Per core the MLP runs
feature-major (activations h^T [feat, batch]).  The two 1024x1024 hidden
layers and the final dot product run in fp8 e4m3 with
perf_mode=DoubleRow: weight k-tile PAIRS are folded into one matmul
(lhsT [128, 2, M], rhs [128, 2, N]) so each PE cell does two multiplies
per cycle.  DoubleRow has no shadow weight buffer, so its 256-column
LDWEIGHTS serializes with the matmul stream (~379 ns/MM at N=512); to
amortize it the batch chunks are processed in GROUPS of 4 with the
weight tile stationary across the group (c-innermost loop).  L1 (K=8)
stays fp32r with 4-way row-group packing.  ReLU+bias runs on ACT for
L1/L2 and on DVE for L3 so the activation work does not gate the PE.
fp8 is safe here: the raw MLP output is ~|0.02| while the physics clamp
floors at 5.0, so the final output is decided entirely by the fp32
constraint path.  The physics-constraint clamp is computed batch-major
on [128, 32] tiles from a host-pretransposed copy of x; the raw MLP
output [1, 512] per chunk is bounced through DRAM to convert it to the
same batch-major layout.
"""

import ml_dtypes
import numpy as np

import concourse.bacc as bacc
import concourse.mybir as mybir
import concourse.tile as tile
from concourse.bass_utils import run_bass_kernel_spmd

F32 = mybir.dt.float32
F32R = mybir.dt.float32r
F8 = mybir.dt.float8e4
NP_F8 = ml_dtypes.float8_e4m3
AF = mybir.ActivationFunctionType
ALU = mybir.AluOpType
DR = mybir.MatmulPerfMode.DoubleRow

N_CORES = 8
B = 32768
BC = B // N_CORES  # 4096 rows per core
D_IN = 8
H = 1024
P = 128
NB = 512  # batch columns per chunk (= one fp32 PSUM bank)
NCH = BC // NB  # 8 chunks per core
KT = H // P  # 8 feature tiles
NPAIR = KT // 2  # 4 DoubleRow k-tile pairs
JT = BC // P  # 32 batch-major columns
G = 4  # chunks per weight-stationary group
NG = NCH // G

_CACHE = {}


def _build_nc():
    nc = bacc.Bacc("TRN2", target_bir_lowering=False, debug=False)

    xT = nc.declare_dram_parameter("xT", [D_IN, BC], F32R, isOutput=False)
    xc = nc.declare_dram_parameter("xc", [P, D_IN * JT], F32, isOutput=False)
    w1 = nc.declare_dram_parameter("w1", [D_IN, H], F32R, isOutput=False)
    w2 = nc.declare_dram_parameter("w2", [P, KT * H], F8, isOutput=False)
    w3 = nc.declare_dram_parameter("w3", [P, KT * H], F8, isOutput=False)
    w4 = nc.declare_dram_parameter("w4", [P, KT * 16], F8, isOutput=False)
    b1 = nc.declare_dram_parameter("b1", [P, KT], F32, isOutput=False)
    b2 = nc.declare_dram_parameter("b2", [P, KT], F32, isOutput=False)
    b3 = nc.declare_dram_parameter("b3", [P, KT], F32, isOutput=False)
    b4 = nc.declare_dram_parameter("b4", [P, 1], F32, isOutput=False)
    out_d = nc.declare_dram_parameter("out_bm", [P, JT], F32, isOutput=True)

    # raw MLP output (pre-b4), bounced through DRAM for the layout flip;
    # exposed as an output so the harness can validate the MLP directly
    # (the physics clamp would otherwise hide any MLP wiring bug).
    raw_scratch = nc.declare_dram_parameter("raw_dbg", [NCH, NB], F32, isOutput=True)

    with tile.TileContext(nc) as tc:
        with (
            tc.tile_pool(name="wts", bufs=1) as wp,
            tc.tile_pool(name="xin", bufs=1) as xp,
            tc.tile_pool(name="acts", bufs=1) as hp,
            tc.tile_pool(name="raw", bufs=2) as rp,
            tc.tile_pool(name="cst", bufs=1) as cp,
            tc.tile_pool(name="ps", bufs=7, space="PSUM") as pp,
            tc.tile_pool(name="ps4", bufs=1, space="PSUM") as pp4,
        ):
            # ---- w1+b1+xT first on the sync queue: L1 is the only PE
            # work available while W2/W3 stream in, so its inputs must
            # land first.
            w1_sb = wp.tile([P, H], F32R, tag="w1")
            nc.sync.dma_start(w1_sb[:D_IN, :], w1[:])
            b1_sb = wp.tile([P, KT], F32, tag="b1")
            nc.sync.dma_start(b1_sb[:], b1[:])
            xt_sb = xp.tile([P, BC], F32R, tag="xt")
            # chunk 0's columns land as their own small transfer so L1(0)
            # isn't gated on the whole 128KB of x.
            nc.sync.dma_start(xt_sb[:D_IN, :NB], xT[:, :NB])
            nc.sync.dma_start(xt_sb[:D_IN, NB:], xT[:, NB:])
            # replicate x / W1 to partition rows 32/64/96 on-chip for the
            # row-group packed L1 of chunks >= 2.
            for i in range(1, 4):
                r0 = 32 * i
                nc.gpsimd.dma_start(w1_sb[r0 : r0 + D_IN, :], w1_sb[:D_IN, :])
                nc.gpsimd.dma_start(xt_sb[r0 : r0 + D_IN, :], xt_sb[:D_IN, :])

            # ---- resident weights/biases -------------------------------
            b2_sb = wp.tile([P, KT], F32, tag="b2")
            nc.gpsimd.dma_start(b2_sb[:], b2[:])
            b3_sb = wp.tile([P, KT], F32, tag="b3")
            nc.gpsimd.dma_start(b3_sb[:], b3[:])
            b4_sb = wp.tile([P, 1], F32, tag="b4")
            nc.gpsimd.dma_start(b4_sb[:], b4[:])
            # w4 padded to [P, KT, 16] so the DoubleRow pair slice
            # [:, 2j:2j+2, 0:1] has a 16-byte middle-dim stride.
            w4_sb = wp.tile([P, KT, 16], F8, tag="w4")
            nc.gpsimd.dma_start(w4_sb[:], w4[:])
            # w2 then w3 on the sync queue, strictly after w1/b1/xT, one
            # k-tile slice at a time so L2(group 0) can start as soon as
            # the first pair lands.
            w2_sb = wp.tile([P, KT, H], F8, tag="w2")
            for k in range(KT):
                nc.sync.dma_start(w2_sb[:, k, :], w2[:, k * H : (k + 1) * H])
            w3_sb = wp.tile([P, KT, H], F8, tag="w3")
            for k in range(KT):
                nc.sync.dma_start(w3_sb[:, k, :], w3[:, k * H : (k + 1) * H])

            # ---- constraint bounds from x (independent of the MLP) -----
            xc_sb = cp.tile([P, D_IN * JT], F32, tag="xc")
            nc.gpsimd.dma_start(xc_sb[:], xc[:])

            def col(c):
                return xc_sb[:, c * JT : (c + 1) * JT]

            cem, slag, fly, wat, ager = col(0), col(1), col(2), col(3), col(7)

            def ctile(name):
                return cp.tile([P, JT], F32, tag=name, name=name)

            def mtile(name):
                return cp.tile([P, JT], mybir.dt.uint8, tag=name, name=name)

            vec = nc.vector

            age = ctile("age")
            vec.tensor_single_scalar(age[:], ager, 1.0, ALU.max)
            cmask = mtile("cmask")
            vec.tensor_single_scalar(cmask[:], cem, 0.0, ALU.is_gt)
            wmask = mtile("wmask")
            vec.tensor_single_scalar(wmask[:], wat, 0.0, ALU.is_gt)
            vmask = mtile("vmask")
            vec.tensor_tensor(vmask[:], cmask[:], wmask[:], ALU.bitwise_and)
            ones = ctile("ones")
            vec.memset(ones[:], 1.0)
            cems = ctile("cems")
            vec.select(cems[:], cmask[:], cem, ones[:])
            rcem = ctile("rcem")
            vec.reciprocal(rcem[:], cems[:])
            wc = ctile("wc")
            vec.tensor_tensor(wc[:], wat, rcem[:], ALU.mult)
            scm = ctile("scm")
            vec.tensor_tensor(scm[:], slag, fly, ALU.add)
            binder = ctile("binder")
            vec.tensor_tensor(binder[:], cem, scm[:], ALU.add)
            den1 = ctile("den1")
            vec.tensor_single_scalar(den1[:], binder[:], 0.1, ALU.max)
            rden1 = ctile("rden1")
            vec.reciprocal(rden1[:], den1[:])
            r1s = ctile("r1s")
            vec.tensor_tensor(r1s[:], scm[:], rden1[:], ALU.mult)
            amax = ctile("amax")
            vec.tensor_scalar(amax[:], r1s[:], -0.15, 0.95, ALU.mult, ALU.add)
            hyd = ctile("hyd")
            vec.tensor_single_scalar(hyd[:], wc[:], 1.0, ALU.add)
            rhyd = ctile("rhyd")
            vec.reciprocal(rhyd[:], hyd[:])
            ea = ctile("ea")
            vec.tensor_tensor(ea[:], rhyd[:], age[:], ALU.mult)
            ex = ctile("ex")
            nc.scalar.activation(ex[:], ea[:], AF.Exp, scale=-0.01)
            omex = ctile("omex")
            vec.tensor_scalar(omex[:], ex[:], -1.0, 1.0, ALU.mult, ALU.add)
            alpha = ctile("alpha")
            vec.tensor_tensor(alpha[:], amax[:], omex[:], ALU.mult)
            bmask = mtile("bmask")
            vec.tensor_single_scalar(bmask[:], binder[:], 0.0, ALU.is_gt)
            bsafe = ctile("bsafe")
            vec.select(bsafe[:], bmask[:], binder[:], ones[:])
            rbs = ctile("rbs")
            vec.reciprocal(rbs[:], bsafe[:])
            cf = ctile("cf")
            vec.tensor_tensor(cf[:], cem, rbs[:], ALU.mult)
            acf = ctile("acf")
            vec.tensor_tensor(acf[:], alpha[:], cf[:], ALU.mult)
            wcmask = mtile("wcmask")
            vec.tensor_single_scalar(wcmask[:], wc[:], 0.0, ALU.is_gt)
            wcsafe = ctile("wcsafe")
            vec.select(wcsafe[:], wcmask[:], wc[:], ones[:])
            rwcs = ctile("rwcs")
            vec.reciprocal(rwcs[:], wcsafe[:])
            gel = ctile("gel")
            vec.tensor_tensor(gel[:], acf[:], rwcs[:], ALU.mult)
            g = ctile("g")
            vec.tensor_scalar(g[:], gel[:], 0.01, 10.0, ALU.max, ALU.min)
            g2 = ctile("g2")
            vec.tensor_tensor(g2[:], g[:], g[:], ALU.mult)
            g3 = ctile("g3")
            vec.tensor_tensor(g3[:], g2[:], g[:], ALU.mult)
            phys = ctile("phys")
            vec.tensor_scalar(phys[:], g3[:], 50.0, 5.0, ALU.mult, ALU.max)
            physl = ctile("physl")
            vec.tensor_single_scalar(physl[:], phys[:], 120.0, ALU.min)
            tot1 = ctile("tot1")
            vec.tensor_tensor(tot1[:], cem, wat, ALU.add)
            total = ctile("total")
            vec.tensor_tensor(total[:], tot1[:], scm[:], ALU.add)
            dtot = ctile("dtot")
            vec.tensor_single_scalar(dtot[:], total[:], 1e-6, ALU.max)
            rtot = ctile("rtot")
            vec.reciprocal(rtot[:], dtot[:])
            cfac = ctile("cfac")
            vec.tensor_tensor(cfac[:], cem, rtot[:], ALU.mult)
            cons = ctile("cons")
            vec.tensor_single_scalar(cons[:], cfac[:], 120.0, ALU.mult)
            ub = ctile("ub")
            vec.tensor_tensor(ub[:], physl[:], cons[:], ALU.min)
            amask = mtile("amask")
            vec.tensor_tensor(amask[:], vmask[:], bmask[:], ALU.bitwise_and)

            # ---- MLP, feature-major, weight-stationary chunk groups ----
            def hpair(layer, c, j, bufs):
                return hp.tile(
                    [P, 2, NB], F8, tag=f"h{layer}p", name=f"h{layer}p_{c}_{j}",
                    bufs=bufs,
                )

            def emit_l1(c, h1):
                # x and W1 are replicated at partitions {0,32,64,96}:
                # chunks >= 2 pack 4 K=8 matmuls into the 4 PE row-groups
                # concurrently (tile_position); chunks 0-1 stay sequential
                # so the PE has steady work while the W2 stream lands.
                packed = c >= 2
                grp = 4 if packed else 1
                for g in range(KT // grp):
                    pss = []
                    for i in range(grp):
                        m = g * grp + i
                        r0 = 32 * i
                        ps = pp.tile([P, NB], F32, tag="ps", name=f"ps1_{c}_{m}")
                        nc.tensor.matmul(
                            ps[:],
                            w1_sb[r0 : r0 + D_IN, m * P : (m + 1) * P],
                            xt_sb[r0 : r0 + D_IN, c * NB : (c + 1) * NB],
                            start=True,
                            stop=True,
                            tile_position=(r0, 0) if packed else None,
                        )
                        pss.append(ps)
                    for i in range(grp):
                        m = g * grp + i
                        nc.scalar.activation(
                            h1[m // 2][:, m % 2, :],
                            pss[i][:],
                            AF.Relu,
                            bias=b1_sb[:, m : m + 1],
                        )

            def hidden_layer(w_sb, b_sb, hin, hout, act, gi, li):
                # weight-stationary: the (m, j) weight tile feeds all G
                # chunks of the group before the PE moves to the next
                # tile, so DoubleRow's serial 256-col LDWEIGHTS amortizes
                # over G matmuls.
                for m in range(KT):
                    pss = []
                    for c in range(G):
                        pss.append(
                            pp.tile(
                                [P, NB], F32, tag="ps", name=f"ps{li}_{gi}_{m}_{c}"
                            )
                        )
                    for j in range(NPAIR):
                        for c in range(G):
                            nc.tensor.matmul(
                                pss[c][:],
                                w_sb[:, 2 * j : 2 * j + 2, m * P : (m + 1) * P],
                                hin[c][j][:],
                                start=(j == 0),
                                stop=(j == NPAIR - 1),
                                perf_mode=DR,
                            )
                    for c in range(G):
                        act(pss[c], m, c)

            raw_bm = cp.tile([P, JT], F32, tag="raw_bm")
            rawb = ctile("rawb")
            lo5 = ctile("lo5")
            constr = ctile("constr")
            outsb = cp.tile([P, JT], F32, tag="outsb")

            nj = NB // P  # batch-major columns produced per chunk

            def raw_to_out(c, ps_part, cols, scr, part_id, eng=None):
                # psum [1, w] -> DRAM bounce -> batch-major columns of
                # raw_bm -> clamp -> store, for a slice of chunk c.
                eng = eng or nc.sync
                w = cols.stop - cols.start
                rawt = rp.tile([1, w], F32, tag="rawt", name=f"rawt{c}_{part_id}")
                vec.tensor_copy(rawt[:], ps_part)
                eng.dma_start(scr, rawt[:])
                sl = slice(c * nj + cols.start // P, c * nj + cols.stop // P)
                eng.dma_start(
                    raw_bm[:, sl],
                    scr.rearrange("c (j p) -> p (c j)", p=P),
                )
                vec.tensor_single_scalar(
                    rawb[:, sl], raw_bm[:, sl], b4_sb[:, 0:1], ALU.add
                )
                vec.tensor_single_scalar(lo5[:, sl], rawb[:, sl], 5.0, ALU.max)
                vec.tensor_tensor(constr[:, sl], lo5[:, sl], ub[:, sl], ALU.min)
                vec.select(outsb[:, sl], amask[:, sl], constr[:, sl], rawb[:, sl])
                nc.gpsimd.dma_start(out_d[:, sl], outsb[:, sl])

            for gi in range(NG):
                cs = [gi * G + i for i in range(G)]

                h1 = [[hpair(1, c, j, 32) for j in range(NPAIR)] for c in cs]
                for i, c in enumerate(cs):
                    emit_l1(c, h1[i])

                h2 = [[hpair(2, c, j, 24) for j in range(NPAIR)] for c in cs]
                hidden_layer(
                    w2_sb,
                    b2_sb,
                    h1,
                    h2,
                    lambda ps, m, c: nc.scalar.activation(
                        h2[c][m // 2][:, m % 2, :],
                        ps[:],
                        AF.Relu,
                        bias=b2_sb[:, m : m + 1],
                    ),
                    gi,
                    2,
                )

                h3 = [[hpair(3, c, j, 24) for j in range(NPAIR)] for c in cs]
                hidden_layer(
                    w3_sb,
                    b3_sb,
                    h2,
                    h3,
                    # L3's ReLU runs on DVE so ACT (busy with L1+L2) does
                    # not gate the PE.
                    lambda ps, m, c: vec.tensor_scalar(
                        h3[c][m // 2][:, m % 2, :],
                        ps[:],
                        b3_sb[:, m : m + 1],
                        0.0,
                        ALU.add,
                        ALU.max,
                    ),
                    gi,
                    3,
                )

                for i, c in enumerate(cs):
                    last = c == NCH - 1
                    if not last:
                        # alternate the L4 bank between the dedicated pp4
                        # bank and the big pool so consecutive chunks'
                        # L4//raw-drain don't serialize on one bank.
                        pool = pp4 if c % 2 == 0 else pp
                        tag = "ps4" if c % 2 == 0 else "ps"
                        ps4 = pool.tile([1, NB], F32, tag=tag, name=f"ps4_{c}")
                        for j in range(NPAIR):
                            nc.tensor.matmul(
                                ps4[:],
                                w4_sb[:, 2 * j : 2 * j + 2, 0:1],
                                h3[i][j][:],
                                start=(j == 0),
                                stop=(j == NPAIR - 1),
                                perf_mode=DR,
                            )
                        raw_to_out(
                            c, ps4[:], slice(0, NB), raw_scratch[c : c + 1, :], "a"
                        )
                    else:
                        # last chunk: L4 split into two half-width
                        # accumulation groups so the first half's slow raw
                        # conversion overlaps the second half's matmuls
                        # instead of trailing them.
                        HB = NB // 2
                        ps4a = pp4.tile([1, HB], F32, tag="ps4", name="ps4_la")
                        ps4b = pp.tile([1, HB], F32, tag="ps", name="ps4_lb")
                        for j in range(NPAIR):
                            nc.tensor.matmul(
                                ps4a[:],
                                w4_sb[:, 2 * j : 2 * j + 2, 0:1],
                                h3[i][j][:, :, :HB],
                                start=(j == 0),
                                stop=(j == NPAIR - 1),
                                perf_mode=DR,
                            )
                        raw_to_out(
                            c, ps4a[:], slice(0, HB), raw_scratch[c : c + 1, :HB], "a"
                        )
                        for j in range(NPAIR):
                            nc.tensor.matmul(
                                ps4b[:],
                                w4_sb[:, 2 * j : 2 * j + 2, 0:1],
                                h3[i][j][:, :, HB:],
                                start=(j == 0),
                                stop=(j == NPAIR - 1),
                                perf_mode=DR,
                            )
                        raw_to_out(
                            c,
                            ps4b[:],
                            slice(HB, NB),
                            raw_scratch[c : c + 1, HB:],
                            "b",
                            eng=nc.gpsimd,
                        )

    nc.compile()
    return nc


def _get_nc():
    if "nc" not in _CACHE:
        _CACHE["nc"] = _build_nc()
    return _CACHE["nc"]


def _prep_in_maps(x, W1, b1, W2, b2, W3, b3, W4, b4):
    f = np.float32
    x = np.ascontiguousarray(np.asarray(x, f))
    W1 = np.ascontiguousarray(np.asarray(W1, f))
    W2 = np.asarray(W2, f)
    W3 = np.asarray(W3, f)
    W4 = np.asarray(W4, f)
    # fp8 weights, k-tile-major layout: w[p, kt, m] = W[kt*128 + p, m]
    w2p = np.ascontiguousarray(
        W2.reshape(KT, P, H).transpose(1, 0, 2).reshape(P, KT * H).astype(NP_F8)
    )
    w3p = np.ascontiguousarray(
        W3.reshape(KT, P, H).transpose(1, 0, 2).reshape(P, KT * H).astype(NP_F8)
    )
    w4p = np.zeros((P, KT, 16), NP_F8)
    w4p[:, :, 0] = W4.reshape(KT, P).T.astype(NP_F8)
    w4p = np.ascontiguousarray(w4p.reshape(P, KT * 16))
    b1p = np.ascontiguousarray(np.asarray(b1, f).reshape(KT, P).T)
    b2p = np.ascontiguousarray(np.asarray(b2, f).reshape(KT, P).T)
    b3p = np.ascontiguousarray(np.asarray(b3, f).reshape(KT, P).T)
    b4p = np.full((P, 1), np.asarray(b4, f).reshape(-1)[0], f)

    in_maps = []
    for c in range(N_CORES):
        sl = x[c * BC : (c + 1) * BC]  # [4096, 8]
        xT_c = np.ascontiguousarray(sl.T)  # [8, 4096]
        # xc[p, col*JT + j] = sl[j*128 + p, col]
        xc_c = np.ascontiguousarray(
            sl.reshape(JT, P, D_IN).transpose(1, 2, 0).reshape(P, D_IN * JT)
        )
        in_maps.append(
            {
                "xT": xT_c,
                "xc": xc_c,
                "w1": W1,
                "w2": w2p,
                "w3": w3p,
                "w4": w4p,
                "b1": b1p,
                "b2": b2p,
                "b3": b3p,
                "b4": b4p,
            }
        )
    return in_maps


def kernel(x, W1, b1, W2, b2, W3, b3, W4, b4, **run_kwargs):
    nc = _get_nc()
    in_maps = _prep_in_maps(x, W1, b1, W2, b2, W3, b3, W4, b4)
    res = run_bass_kernel_spmd(nc, in_maps, core_ids=list(range(N_CORES)), **run_kwargs)
    out = np.empty((B, 1), np.float32)
    for c in range(N_CORES):
        out[c * BC : (c + 1) * BC, 0] = res.results[c]["out_bm"].T.reshape(BC)
    if run_kwargs:
        kernel.last_results = res
    return out


# revision 4
# speedup vs baseline: 1.0485x; 1.0485x over previous
"""Trainium2 Bass kernel for the CementPINN MLP (dense_mlp, 8 cores).

Data-parallel: x [32768, 8] is sharded along batch into 8 shards of 4096
rows; MLP weights are replicated on every core.  Per core the MLP runs
feature-major (activations h^T [feat, batch]).  ALL matmuls run in fp8
e4m3 with perf_mode=DoubleRow: k-tile PAIRS are folded into one matmul
(lhsT [K, 2, M], rhs [K, 2, N]) so each PE cell does two multiplies per
cycle -- the measured issue rate is ~216 ns per N=512 DoubleRow matmul,
i.e. 2x the fp32r rate per MAC.  L1 folds its K=8 contraction into 4
partitions x 2.  ReLU+bias runs on ACT for L1/L2 and on DVE for L3 so
the activation work does not gate the PE.  fp8 is safe here: the raw
MLP output is ~|0.02| while the physics clamp floors at 5.0, so the
final output is decided entirely by the fp32 constraint path.
The physics-constraint clamp is computed batch-major on [128, 32] tiles
from a host-pretransposed fp32 copy of x; the raw MLP output [1, 512]
per chunk is bounced through DRAM to convert it to the same batch-major
layout.
"""

import ml_dtypes
import numpy as np

import concourse.bacc as bacc
import concourse.mybir as mybir
import concourse.tile as tile
from concourse.bass_utils import run_bass_kernel_spmd

F32 = mybir.dt.float32
F8 = mybir.dt.float8e4
NP_F8 = ml_dtypes.float8_e4m3
AF = mybir.ActivationFunctionType
ALU = mybir.AluOpType
DR = mybir.MatmulPerfMode.DoubleRow

N_CORES = 8
B = 32768
BC = B // N_CORES  # 4096 rows per core
D_IN = 8
H = 1024
P = 128
NB = 512  # batch columns per chunk (= one fp32 PSUM bank)
NCH = BC // NB  # 8 chunks per core
KT = H // P  # 8 feature tiles
NPAIR = KT // 2  # 4 DoubleRow k-tile pairs
JT = BC // P  # 32 batch-major columns
M4 = 16  # L4 stationary padded to 16 output columns (row 0 is real)

_CACHE = {}


def _build_nc():
    nc = bacc.Bacc("TRN2", target_bir_lowering=False, debug=False)

    xT = nc.declare_dram_parameter("xT", [4, 2 * BC], F8, isOutput=False)
    xc = nc.declare_dram_parameter("xc", [P, D_IN * JT], F32, isOutput=False)
    w1 = nc.declare_dram_parameter("w1", [4, 2 * H], F8, isOutput=False)
    w2 = nc.declare_dram_parameter("w2", [P, KT * H], F8, isOutput=False)
    w3 = nc.declare_dram_parameter("w3", [P, KT * H], F8, isOutput=False)
    w4 = nc.declare_dram_parameter("w4", [P, KT * M4], F8, isOutput=False)
    b1 = nc.declare_dram_parameter("b1", [P, KT], F32, isOutput=False)
    b2 = nc.declare_dram_parameter("b2", [P, KT], F32, isOutput=False)
    b3 = nc.declare_dram_parameter("b3", [P, KT], F32, isOutput=False)
    b4 = nc.declare_dram_parameter("b4", [P, 1], F32, isOutput=False)
    out_d = nc.declare_dram_parameter("out_bm", [P, JT], F32, isOutput=True)

    # raw MLP output (pre-b4), bounced through DRAM for the layout flip;
    # exposed as an output so the harness can validate the MLP directly
    # (the physics clamp would otherwise hide any MLP wiring bug).
    raw_scratch = nc.declare_dram_parameter("raw_dbg", [NCH, NB], F32, isOutput=True)

    with tile.TileContext(nc) as tc:
        with (
            tc.tile_pool(name="wts", bufs=1) as wp,
            tc.tile_pool(name="xin", bufs=1) as xp,
            tc.tile_pool(name="acts", bufs=1) as hp,
            tc.tile_pool(name="raw", bufs=2) as rp,
            tc.tile_pool(name="cst", bufs=1) as cp,
            tc.tile_pool(name="ps", bufs=7, space="PSUM") as pp,
            tc.tile_pool(name="ps4", bufs=1, space="PSUM") as pp4,
        ):
            # ---- L1 inputs first on the sync queue (tiny in fp8), then
            # the W2/W3 stream one k-slice at a time so L2(0) can start
            # as soon as the first pair lands.
            w1_sb = wp.tile([4, 2, H], F8, tag="w1")
            nc.sync.dma_start(w1_sb[:], w1[:])
            b1_sb = wp.tile([P, KT], F32, tag="b1")
            nc.sync.dma_start(b1_sb[:], b1[:])
            xt_sb = xp.tile([4, 2, BC], F8, tag="xt")
            nc.sync.dma_start(xt_sb[:], xT[:])
            w2_sb = wp.tile([P, KT, H], F8, tag="w2")
            for k in range(KT):
                nc.sync.dma_start(w2_sb[:, k, :], w2[:, k * H : (k + 1) * H])
            w3_sb = wp.tile([P, KT, H], F8, tag="w3")
            for k in range(KT):
                nc.sync.dma_start(w3_sb[:, k, :], w3[:, k * H : (k + 1) * H])

            # ---- resident weights/biases (gpsimd queue) ----------------
            b2_sb = wp.tile([P, KT], F32, tag="b2")
            nc.gpsimd.dma_start(b2_sb[:], b2[:])
            b3_sb = wp.tile([P, KT], F32, tag="b3")
            nc.gpsimd.dma_start(b3_sb[:], b3[:])
            b4_sb = wp.tile([P, 1], F32, tag="b4")
            nc.gpsimd.dma_start(b4_sb[:], b4[:])
            # w4 padded to [P, KT, M4] (column 0 real, rest zero) so the
            # DoubleRow pair slice has a 16-byte middle-dim stride and a
            # full 16-column stationary.
            w4_sb = wp.tile([P, KT, M4], F8, tag="w4")
            nc.gpsimd.dma_start(w4_sb[:], w4[:])

            # ---- constraint bounds from x (independent of the MLP) -----
            xc_sb = cp.tile([P, D_IN * JT], F32, tag="xc")
            nc.gpsimd.dma_start(xc_sb[:], xc[:])

            def col(c):
                return xc_sb[:, c * JT : (c + 1) * JT]

            cem, slag, fly, wat, ager = col(0), col(1), col(2), col(3), col(7)

            def ctile(name):
                return cp.tile([P, JT], F32, tag=name, name=name)

            def mtile(name):
                return cp.tile([P, JT], mybir.dt.uint8, tag=name, name=name)

            vec = nc.vector

            age = ctile("age")
            vec.tensor_single_scalar(age[:], ager, 1.0, ALU.max)
            cmask = mtile("cmask")
            vec.tensor_single_scalar(cmask[:], cem, 0.0, ALU.is_gt)
            wmask = mtile("wmask")
            vec.tensor_single_scalar(wmask[:], wat, 0.0, ALU.is_gt)
            vmask = mtile("vmask")
            vec.tensor_tensor(vmask[:], cmask[:], wmask[:], ALU.bitwise_and)
            ones = ctile("ones")
            vec.memset(ones[:], 1.0)
            cems = ctile("cems")
            vec.select(cems[:], cmask[:], cem, ones[:])
            rcem = ctile("rcem")
            vec.reciprocal(rcem[:], cems[:])
            wc = ctile("wc")
            vec.tensor_tensor(wc[:], wat, rcem[:], ALU.mult)
            scm = ctile("scm")
            vec.tensor_tensor(scm[:], slag, fly, ALU.add)
            binder = ctile("binder")
            vec.tensor_tensor(binder[:], cem, scm[:], ALU.add)
            den1 = ctile("den1")
            vec.tensor_single_scalar(den1[:], binder[:], 0.1, ALU.max)
            rden1 = ctile("rden1")
            vec.reciprocal(rden1[:], den1[:])
            r1s = ctile("r1s")
            vec.tensor_tensor(r1s[:], scm[:], rden1[:], ALU.mult)
            amax = ctile("amax")
            vec.tensor_scalar(amax[:], r1s[:], -0.15, 0.95, ALU.mult, ALU.add)
            hyd = ctile("hyd")
            vec.tensor_single_scalar(hyd[:], wc[:], 1.0, ALU.add)
            rhyd = ctile("rhyd")
            vec.reciprocal(rhyd[:], hyd[:])
            ea = ctile("ea")
            vec.tensor_tensor(ea[:], rhyd[:], age[:], ALU.mult)
            ex = ctile("ex")
            nc.scalar.activation(ex[:], ea[:], AF.Exp, scale=-0.01)
            omex = ctile("omex")
            vec.tensor_scalar(omex[:], ex[:], -1.0, 1.0, ALU.mult, ALU.add)
            alpha = ctile("alpha")
            vec.tensor_tensor(alpha[:], amax[:], omex[:], ALU.mult)
            bmask = mtile("bmask")
            vec.tensor_single_scalar(bmask[:], binder[:], 0.0, ALU.is_gt)
            bsafe = ctile("bsafe")
            vec.select(bsafe[:], bmask[:], binder[:], ones[:])
            rbs = ctile("rbs")
            vec.reciprocal(rbs[:], bsafe[:])
            cf = ctile("cf")
            vec.tensor_tensor(cf[:], cem, rbs[:], ALU.mult)
            acf = ctile("acf")
            vec.tensor_tensor(acf[:], alpha[:], cf[:], ALU.mult)
            wcmask = mtile("wcmask")
            vec.tensor_single_scalar(wcmask[:], wc[:], 0.0, ALU.is_gt)
            wcsafe = ctile("wcsafe")
            vec.select(wcsafe[:], wcmask[:], wc[:], ones[:])
            rwcs = ctile("rwcs")
            vec.reciprocal(rwcs[:], wcsafe[:])
            gel = ctile("gel")
            vec.tensor_tensor(gel[:], acf[:], rwcs[:], ALU.mult)
            g = ctile("g")
            vec.tensor_scalar(g[:], gel[:], 0.01, 10.0, ALU.max, ALU.min)
            g2 = ctile("g2")
            vec.tensor_tensor(g2[:], g[:], g[:], ALU.mult)
            g3 = ctile("g3")
            vec.tensor_tensor(g3[:], g2[:], g[:], ALU.mult)
            phys = ctile("phys")
            vec.tensor_scalar(phys[:], g3[:], 50.0, 5.0, ALU.mult, ALU.max)
            physl = ctile("physl")
            vec.tensor_single_scalar(physl[:], phys[:], 120.0, ALU.min)
            tot1 = ctile("tot1")
            vec.tensor_tensor(tot1[:], cem, wat, ALU.add)
            total = ctile("total")
            vec.tensor_tensor(total[:], tot1[:], scm[:], ALU.add)
            dtot = ctile("dtot")
            vec.tensor_single_scalar(dtot[:], total[:], 1e-6, ALU.max)
            rtot = ctile("rtot")
            vec.reciprocal(rtot[:], dtot[:])
            cfac = ctile("cfac")
            vec.tensor_tensor(cfac[:], cem, rtot[:], ALU.mult)
            cons = ctile("cons")
            vec.tensor_single_scalar(cons[:], cfac[:], 120.0, ALU.mult)
            ub = ctile("ub")
            vec.tensor_tensor(ub[:], physl[:], cons[:], ALU.min)
            amask = mtile("amask")
            vec.tensor_tensor(amask[:], vmask[:], bmask[:], ALU.bitwise_and)

            # ---- MLP, feature-major, chunked over batch columns --------
            def hpair(layer, c, j, bufs=12):
                return hp.tile(
                    [P, 2, NB], F8, tag=f"h{layer}p", name=f"h{layer}p_{c}_{j}",
                    bufs=bufs,
                )

            raw_bm = cp.tile([P, JT], F32, tag="raw_bm")
            rawb = ctile("rawb")
            lo5 = ctile("lo5")
            constr = ctile("constr")
            outsb = cp.tile([P, JT], F32, tag="outsb")

            nj = NB // P  # batch-major columns produced per chunk

            def raw_to_out(c, ps_part, cols, scr, part_id, eng=None):
                # psum [1, w] -> DRAM bounce -> batch-major columns of
                # raw_bm -> clamp -> store, for a slice of chunk c.
                eng = eng or nc.sync
                w = cols.stop - cols.start
                rawt = rp.tile([1, w], F32, tag="rawt", name=f"rawt{c}_{part_id}")
                vec.tensor_copy(rawt[:], ps_part)
                eng.dma_start(scr, rawt[:])
                sl = slice(c * nj + cols.start // P, c * nj + cols.stop // P)
                eng.dma_start(
                    raw_bm[:, sl],
                    scr.rearrange("c (j p) -> p (c j)", p=P),
                )
                vec.tensor_single_scalar(
                    rawb[:, sl], raw_bm[:, sl], b4_sb[:, 0:1], ALU.add
                )
                vec.tensor_single_scalar(lo5[:, sl], rawb[:, sl], 5.0, ALU.max)
                vec.tensor_tensor(constr[:, sl], lo5[:, sl], ub[:, sl], ALU.min)
                vec.select(outsb[:, sl], amask[:, sl], constr[:, sl], rawb[:, sl])
                nc.gpsimd.dma_start(out_d[:, sl], outsb[:, sl])

            for c in range(NCH):
                # L1: K=8 folded into one DoubleRow matmul per m-tile.
                h1 = [hpair(1, c, j) for j in range(NPAIR)]
                for m in range(KT):
                    ps = pp.tile([P, NB], F32, tag="ps", name=f"ps1_{c}_{m}")
                    nc.tensor.matmul(
                        ps[:],
                        w1_sb[:, :, m * P : (m + 1) * P],
                        xt_sb[:, :, c * NB : (c + 1) * NB],
                        start=True,
                        stop=True,
                        perf_mode=DR,
                    )
                    nc.scalar.activation(
                        h1[m // 2][:, m % 2, :], ps[:], AF.Relu,
                        bias=b1_sb[:, m : m + 1],
                    )

                h2 = [hpair(2, c, j) for j in range(NPAIR)]
                for m in range(KT):
                    ps = pp.tile([P, NB], F32, tag="ps", name=f"ps2_{c}_{m}")
                    for j in range(NPAIR):
                        nc.tensor.matmul(
                            ps[:],
                            w2_sb[:, 2 * j : 2 * j + 2, m * P : (m + 1) * P],
                            h1[j][:],
                            start=(j == 0),
                            stop=(j == NPAIR - 1),
                            perf_mode=DR,
                        )
                    nc.scalar.activation(
                        h2[m // 2][:, m % 2, :], ps[:], AF.Relu,
                        bias=b2_sb[:, m : m + 1],
                    )

                h3 = [hpair(3, c, j) for j in range(NPAIR)]
                for m in range(KT):
                    ps = pp.tile([P, NB], F32, tag="ps", name=f"ps3_{c}_{m}")
                    for j in range(NPAIR):
                        nc.tensor.matmul(
                            ps[:],
                            w3_sb[:, 2 * j : 2 * j + 2, m * P : (m + 1) * P],
                            h2[j][:],
                            start=(j == 0),
                            stop=(j == NPAIR - 1),
                            perf_mode=DR,
                        )
                    # L3's ReLU runs on DVE so ACT (busy with L1+L2) does
                    # not gate the PE.
                    vec.tensor_scalar(
                        h3[m // 2][:, m % 2, :], ps[:], b3_sb[:, m : m + 1],
                        0.0, ALU.add, ALU.max,
                    )

                last = c == NCH - 1
                if not last:
                    # alternate the L4 bank between the dedicated pp4 bank
                    # and the big pool so consecutive chunks' L4/raw-drain
                    # don't serialize on one bank.
                    pool = pp4 if c % 2 == 0 else pp
                    tag = "ps4" if c % 2 == 0 else "ps"
                    ps4 = pool.tile([M4, NB], F32, tag=tag, name=f"ps4_{c}")
                    for j in range(NPAIR):
                        nc.tensor.matmul(
                            ps4[:],
                            w4_sb[:, 2 * j : 2 * j + 2, :],
                            h3[j][:],
                            start=(j == 0),
                            stop=(j == NPAIR - 1),
                            perf_mode=DR,
                        )
                    raw_to_out(
                        c, ps4[0:1, :], slice(0, NB), raw_scratch[c : c + 1, :], "a"
                    )
                else:
                    # last chunk: L4 split into two half-width accumulation
                    # groups so the first half's slow raw conversion
                    # overlaps the second half's matmuls instead of
                    # trailing them.
                    HB = NB // 2
                    ps4a = pp4.tile([M4, HB], F32, tag="ps4", name="ps4_la")
                    ps4b = pp.tile([M4, HB], F32, tag="ps", name="ps4_lb")
                    for j in range(NPAIR):
                        nc.tensor.matmul(
                            ps4a[:],
                            w4_sb[:, 2 * j : 2 * j + 2, :],
                            h3[j][:, :, :HB],
                            start=(j == 0),
                            stop=(j == NPAIR - 1),
                            perf_mode=DR,
                        )
                    raw_to_out(
                        c, ps4a[0:1, :], slice(0, HB),
                        raw_scratch[c : c + 1, :HB], "a",
                    )
                    for j in range(NPAIR):
                        nc.tensor.matmul(
                            ps4b[:],
                            w4_sb[:, 2 * j : 2 * j + 2, :],
                            h3[j][:, :, HB:],
                            start=(j == 0),
                            stop=(j == NPAIR - 1),
                            perf_mode=DR,
                        )
                    raw_to_out(
                        c, ps4b[0:1, :], slice(HB, NB),
                        raw_scratch[c : c + 1, HB:], "b",
                        eng=nc.gpsimd,
                    )

    nc.compile()
    return nc


def _get_nc():
    if "nc" not in _CACHE:
        _CACHE["nc"] = _build_nc()
    return _CACHE["nc"]


def _prep_in_maps(x, W1, b1, W2, b2, W3, b3, W4, b4):
    f = np.float32
    x = np.ascontiguousarray(np.asarray(x, f))
    W1 = np.asarray(W1, f)
    W2 = np.asarray(W2, f)
    W3 = np.asarray(W3, f)
    W4 = np.asarray(W4, f)
    # L1 DoubleRow fold: feature f = 2p + i -> [4, 2, .] fp8
    w1p = np.ascontiguousarray(W1.reshape(4, 2 * H).astype(NP_F8))
    # fp8 hidden weights, k-tile-major layout: w[p, kt, m] = W[kt*128+p, m]
    w2p = np.ascontiguousarray(
        W2.reshape(KT, P, H).transpose(1, 0, 2).reshape(P, KT * H).astype(NP_F8)
    )
    w3p = np.ascontiguousarray(
        W3.reshape(KT, P, H).transpose(1, 0, 2).reshape(P, KT * H).astype(NP_F8)
    )
    w4p = np.zeros((P, KT, M4), NP_F8)
    w4p[:, :, 0] = W4.reshape(KT, P).T.astype(NP_F8)
    w4p = np.ascontiguousarray(w4p.reshape(P, KT * M4))
    b1p = np.ascontiguousarray(np.asarray(b1, f).reshape(KT, P).T)
    b2p = np.ascontiguousarray(np.asarray(b2, f).reshape(KT, P).T)
    b3p = np.ascontiguousarray(np.asarray(b3, f).reshape(KT, P).T)
    b4p = np.full((P, 1), np.asarray(b4, f).reshape(-1)[0], f)

    in_maps = []
    for c in range(N_CORES):
        sl = x[c * BC : (c + 1) * BC]  # [4096, 8]
        # xT fp8 [4, 2, BC]: x feature f=2p+i at [p, i, :]
        xT_c = np.ascontiguousarray(
            sl.T.reshape(4, 2 * BC).astype(NP_F8)
        )
        # xc[p, col*JT + j] = sl[j*128 + p, col]  (fp32, exact clamp path)
        xc_c = np.ascontiguousarray(
            sl.reshape(JT, P, D_IN).transpose(1, 2, 0).reshape(P, D_IN * JT)
        )
        in_maps.append(
            {
                "xT": xT_c,
                "xc": xc_c,
                "w1": w1p,
                "w2": w2p,
                "w3": w3p,
                "w4": w4p,
                "b1": b1p,
                "b2": b2p,
                "b3": b3p,
                "b4": b4p,
            }
        )
    return in_maps


def kernel(x, W1, b1, W2, b2, W3, b3, W4, b4, **run_kwargs):
    nc = _get_nc()
    in_maps = _prep_in_maps(x, W1, b1, W2, b2, W3, b3, W4, b4)
    res = run_bass_kernel_spmd(nc, in_maps, core_ids=list(range(N_CORES)), **run_kwargs)
    out = np.empty((B, 1), np.float32)
    for c in range(N_CORES):
        out[c * BC : (c + 1) * BC, 0] = res.results[c]["out_bm"].T.reshape(BC)
    if run_kwargs:
        kernel.last_results = res
    return out
